# revision 1
# baseline (speedup 1.0000x reference)
"""Hawk (RG-LRU) block kernel for Trainium2, SPMD over 8 NeuronCores.

Sharding: tokens. Core k handles batch b=k//2, half h=k%2 (2048 tokens).
All weights replicated (host-transposed into matmul-ready layouts).
On-chip layout is channel-major [channel partitions, time free]; the
diagonal recurrence runs as hardware tensor_tensor_scan along the free
dim. The cross-half scan carry moves via a pairwise AllReduce of 4KB.
Matmuls run in float32r (full-rate, ~1.5e-4 rel err).
"""
import sys

sys.path.insert(0, "/opt/trn_rl_repo")

import numpy as np
from contextlib import ExitStack

import concourse.bass as bass
import concourse.tile as tile
import concourse.bacc as bacc
from concourse import mybir
from concourse.bass_utils import run_bass_kernel_spmd

F32 = mybir.dt.float32
F32R = mybir.dt.float32r
BF16 = mybir.dt.bfloat16
AF = mybir.ActivationFunctionType
OP = mybir.AluOpType

B, T, DIM = 4, 4096, 1024
E = 1024
KC = 4  # conv taps
N_CORES = 8
T_LOC = T // 2      # 2048 tokens per core
TT = 512            # token tile
NTT = T_LOC // TT   # 4
NE = E // 128       # 8 channel chunks
NK = DIM // 128     # 8 contraction tiles


def _build_kernel(profile_mode=False):
    nc = bacc.Bacc("TRN2", target_bir_lowering=False, debug=False,
                   num_devices=1 if profile_mode else N_CORES)

    xT = nc.dram_tensor("xT", [DIM, T_LOC], F32, kind="ExternalInput")
    xa_halo = nc.dram_tensor("xa_halo", [E, KC - 1], F32, kind="ExternalInput")
    w_in_g = nc.dram_tensor("w_in_g", [DIM, E], F32, kind="ExternalInput")
    w_in_x = nc.dram_tensor("w_in_x", [DIM, E], F32, kind="ExternalInput")
    w_gates = nc.dram_tensor("w_gates", [E, 2 * E], F32, kind="ExternalInput")
    w_out = nc.dram_tensor("w_out", [E, DIM], F32, kind="ExternalInput")
    wc = nc.dram_tensor("wc", [E, KC], F32, kind="ExternalInput")
    b_conv = nc.dram_tensor("b_conv", [E, 1], F32, kind="ExternalInput")
    neg_c = nc.dram_tensor("neg_c", [E, 1], F32, kind="ExternalInput")
    b_f = nc.dram_tensor("b_f", [E, 1], F32, kind="ExternalInput")
    b_i = nc.dram_tensor("b_i", [E, 1], F32, kind="ExternalInput")
    mask_c = nc.dram_tensor("mask_c", [128, 1], F32, kind="ExternalInput")
    mask_u = nc.dram_tensor("mask_u", [128, 1], F32, kind="ExternalInput")
    out = nc.dram_tensor("out", [T_LOC, DIM], F32, kind="ExternalOutput")

    with tile.TileContext(nc) as tc, ExitStack() as ctx:
        _body(ctx, tc, nc, profile_mode=profile_mode,
              xT=xT, xa_halo=xa_halo, w_in_g=w_in_g,
              w_in_x=w_in_x, w_gates=w_gates, w_out=w_out, wc=wc,
              b_conv=b_conv, neg_c=neg_c, b_f=b_f, b_i=b_i,
              mask_c=mask_c, mask_u=mask_u, out=out)
    nc.compile()
    return nc


def _body(ctx, tc, nc, *, xT, xa_halo, w_in_g, w_in_x, w_gates, w_out, wc,
          b_conv, neg_c, b_f, b_i, mask_c, mask_u, out, profile_mode=False):
    consts = ctx.enter_context(tc.tile_pool(name="consts", bufs=1))
    ps1 = ctx.enter_context(tc.tile_pool(name="ps1", bufs=8, space="PSUM"))
    dram = ctx.enter_context(tc.tile_pool(name="dram", bufs=1, space="DRAM"))

    def load_chan_const(t_dram, n):
        t = consts.tile([128, NE, n], F32, tag=t_dram.name)
        nc.sync.dma_start(t[:], t_dram.ap().rearrange("(m p) n -> p m n", p=128))
        return t

    wc_sb = load_chan_const(wc, KC)
    b_conv_sb = load_chan_const(b_conv, 1)
    neg_c_sb = load_chan_const(neg_c, 1)
    b_f_sb = load_chan_const(b_f, 1)
    b_i_sb = load_chan_const(b_i, 1)
    mc_sb = consts.tile([128, 1], F32, tag="mc")
    nc.sync.dma_start(mc_sb[:], mask_c.ap()[:])
    mu_sb = consts.tile([128, 1], F32, tag="mu")
    nc.sync.dma_start(mu_sb[:], mask_u.ap()[:])
    zeros = consts.tile([128, TT], F32, tag="zeros")
    nc.vector.memset(zeros[:], 0.0)
    c_zero = consts.tile([128, 1], F32, tag="c_zero")
    nc.vector.memset(c_zero[:], 0.0)
    c_sqb = consts.tile([128, 1], F32, tag="c_sqb")
    nc.vector.memset(c_sqb[:], 1.000001)
    hcarry = consts.tile([128, NE], F32, tag="hcarry")
    carry = consts.tile([128, NE], F32, tag="carry")

    h_dram = dram.tile([NE, NTT, 128, TT], F32, tag="h_spill")
    p_dram = dram.tile([NE, NTT, 128, TT], F32, tag="p_spill")
    xc_dram = dram.tile([NTT, 128, NE, TT], F32R, tag="xc_spill")
    cc_in = dram.tile([E], F32, tag="cc_in")
    cc_out = dram.tile([E], F32, tag="cc_out")

    # w_gates loads up-front so it streams in while stage A computes
    wg_stack = ctx.enter_context(ExitStack())
    wg = wg_stack.enter_context(tc.tile_pool(name="w_gates", bufs=1, side="right"))
    wg_sb = wg.tile([128, NK, 2 * E], F32R)
    wg_src = w_gates.ap().rearrange("(k p) f -> p k f", p=128)

    # ============ Stage A: xa proj + causal conv -> xc (spilled) =======
    with ExitStack() as sa:
        wx = sa.enter_context(tc.tile_pool(name="w_in_x", bufs=1, side="right"))
        wx_sb = wx.tile([128, NK, E], F32R)
        wx_src = w_in_x.ap().rearrange("(k p) e -> p k e", p=128)
        xc_pool = sa.enter_context(tc.tile_pool(name="xc", bufs=2))
        xs_pool = sa.enter_context(tc.tile_pool(name="xstream", bufs=16))
        xa_pool = sa.enter_context(tc.tile_pool(name="xa", bufs=2))
        c3_pool = sa.enter_context(tc.tile_pool(name="c3", bufs=2))

        c3prev = c3_pool.tile([128, NE, KC - 1], F32, tag="c3")
        nc.sync.dma_start(c3prev[:],
                          xa_halo.ap().rearrange("(m p) n -> p m n", p=128))
        for tt in range(NTT):
            xtt = []
            for k in range(NK):
                if tt == 0:
                    nc.sync.dma_start(wx_sb[:, k], wx_src[:, k].bitcast(F32R))
                t = xs_pool.tile([128, TT], F32R, tag="xstream")
                nc.sync.dma_start(
                    t[:], xT.ap()[k * 128:(k + 1) * 128,
                                  tt * TT:(tt + 1) * TT].bitcast(F32R))
                xtt.append(t)
            for k in range(2 * tt, 2 * tt + 2):
                nc.sync.dma_start(wg_sb[:, k], wg_src[:, k].bitcast(F32R))
            xat = xa_pool.tile([128, NE, TT], F32, tag="xa")
            c3t = c3_pool.tile([128, NE, KC - 1], F32, tag="c3")
            for m in range(NE):
                pt = ps1.tile([128, TT], F32, tag="ps")
                for k in range(NK):
                    nc.tensor.matmul(pt[:], wx_sb[:, k, m * 128:(m + 1) * 128],
                                     xtt[k][:], start=(k == 0), stop=(k == NK - 1))
                nc.scalar.copy(xat[:, m], pt[:])
                nc.vector.tensor_copy(c3t[:, m], pt[:, TT - KC + 1:TT])
            xct = xc_pool.tile([128, NE, TT], F32R, tag="xc")
            for m in range(NE):
                # tap 0 (+bias), head from carry then main
                nc.vector.tensor_scalar(
                    xct[:, m, 0:KC - 1], c3prev[:, m], wc_sb[:, m, 0:1],
                    b_conv_sb[:, m, 0:1], op0=OP.mult, op1=OP.add)
                nc.vector.tensor_scalar(
                    xct[:, m, KC - 1:TT], xat[:, m, 0:TT - KC + 1],
                    wc_sb[:, m, 0:1], b_conv_sb[:, m, 0:1],
                    op0=OP.mult, op1=OP.add)
                for j in range(1, KC):
                    hw = KC - 1 - j  # head width
                    if hw > 0:
                        nc.vector.scalar_tensor_tensor(
                            xct[:, m, 0:hw], c3prev[:, m, j:KC - 1],
                            wc_sb[:, m, j:j + 1], xct[:, m, 0:hw].bitcast(F32),
                            op0=OP.mult, op1=OP.add)
                    nc.vector.scalar_tensor_tensor(
                        xct[:, m, hw:TT], xat[:, m, 0:TT - hw],
                        wc_sb[:, m, j:j + 1], xct[:, m, hw:TT].bitcast(F32),
                        op0=OP.mult, op1=OP.add)
            nc.sync.dma_start(xc_dram[tt], xct[:])
            c3prev = c3t

    # ============ Stage B: gates + elementwise + scans =================
    with ExitStack() as sb:
        # gate-projection weights load during stage B (used in stage D)
        wgt = sb.enter_context(tc.tile_pool(name="w_in_g", bufs=1))
        wg_in_sb = wgt.tile([128, NK, E], F32R)
        wgi_src = w_in_g.ap().rearrange("(k p) e -> p k e", p=128)
        dpre = sb.enter_context(tc.tile_pool(name="dpre", bufs=1))
        dpre_x = dpre.tile([128, 4, TT], F32R)
        sbw = sb.enter_context(ExitStack())
        xcs_pool = sbw.enter_context(tc.tile_pool(name="xcs", bufs=2))
        work = sbw.enter_context(tc.tile_pool(name="work", bufs=3))
        apool = sbw.enter_context(tc.tile_pool(name="apool", bufs=5))
        upool = sbw.enter_context(tc.tile_pool(name="upool", bufs=3))
        hp = sbw.enter_context(tc.tile_pool(name="hp", bufs=3))
        pp = sbw.enter_context(tc.tile_pool(name="pp", bufs=3))
        lc = sbw.enter_context(tc.tile_pool(name="lc", bufs=1))
        hl = {m: lc.tile([128, 1], F32, tag=f"hl{m}", name=f"hl{m}") for m in range(NE)}
        pl = {m: lc.tile([128, 1], F32, tag=f"pl{m}", name=f"pl{m}") for m in range(NE)}

        for tt in range(NTT):
            xct = xcs_pool.tile([128, NE, TT], F32R, tag="xcs")
            nc.sync.dma_start(xct[:], xc_dram[tt])
            for g4 in range(2):
                ms = range(g4 * 4, g4 * 4 + 4)
                pfs, pis, sfs, sis, alphas, us = {}, {}, {}, {}, {}, {}
                for m in ms:
                    pf = ps1.tile([128, TT], F32, tag="ps")
                    for k in range(NK):
                        nc.tensor.matmul(pf[:], wg_sb[:, k, m * 128:(m + 1) * 128],
                                         xct[:, k], start=(k == 0), stop=(k == NK - 1))
                    pfs[m] = pf
                    pi = ps1.tile([128, TT], F32, tag="ps")
                    for k in range(NK):
                        nc.tensor.matmul(pi[:], wg_sb[:, k, E + m * 128:E + (m + 1) * 128],
                                         xct[:, k], start=(k == 0), stop=(k == NK - 1))
                    pis[m] = pi
                for m in ms:
                    sf = work.tile([128, TT], F32, tag="sf")
                    nc.scalar.activation(sf[:], pfs[m][:], AF.Sigmoid,
                                         bias=b_f_sb[:, m, 0:1])
                    sfs[m] = sf
                    si = work.tile([128, TT], F32, tag="si")
                    nc.scalar.activation(si[:], pis[m][:], AF.Sigmoid,
                                         bias=b_i_sb[:, m, 0:1])
                    sis[m] = si
                for m in ms:
                    alpha = apool.tile([128, TT], F32, tag="alpha")
                    nc.scalar.activation(alpha[:], sfs[m][:], AF.Exp,
                                         scale=neg_c_sb[:, m, 0:1])
                    alphas[m] = alpha
                for m in ms:
                    asq = work.tile([128, TT], F32, tag="asq")
                    nc.vector.tensor_mul(asq[:], alphas[m][:], alphas[m][:])
                    sfs[m] = asq
                for m in ms:
                    nc.scalar.activation(sfs[m][:], sfs[m][:], AF.Sqrt,
                                         bias=c_sqb[:], scale=-1.0)
                for m in ms:
                    bs = work.tile([128, TT], F32, tag="bs")
                    nc.vector.tensor_mul(bs[:], sfs[m][:], sis[m][:])
                    u = upool.tile([128, TT], F32, tag="u")
                    nc.vector.tensor_mul(u[:], bs[:], xct[:, m].bitcast(F32))
                    us[m] = u
                for m in ms:
                    ht = hp.tile([128, TT], F32, tag="h")
                    nc.vector.tensor_tensor_scan(
                        ht[:], alphas[m][:], us[m][:],
                        0.0 if tt == 0 else hl[m][:],
                        op0=OP.mult, op1=OP.add)
                    nc.vector.tensor_copy(hl[m][:], ht[:, TT - 1:TT])
                    pt = pp.tile([128, TT], F32, tag="p")
                    nc.vector.tensor_tensor_scan(
                        pt[:], alphas[m][:], zeros[:],
                        1.0 if tt == 0 else pl[m][:],
                        op0=OP.mult, op1=OP.add)
                    nc.vector.tensor_copy(pl[m][:], pt[:, TT - 1:TT])
                    nc.sync.dma_start(h_dram[m, tt], ht[:])
                    nc.sync.dma_start(p_dram[m, tt], pt[:])
            for k in range(2 * tt, 2 * tt + 2):
                nc.sync.dma_start(wg_in_sb[:, k], wgi_src[:, k].bitcast(F32R))
            if tt == 2:
                for k in range(4):
                    nc.sync.dma_start(
                        dpre_x[:, k],
                        xT.ap()[k * 128:(k + 1) * 128, 0:TT].bitcast(F32R))
        for m in range(NE):
            nc.scalar.copy(hcarry[:, m:m + 1], hl[m][:])
        sbw.close()
        wg_stack.close()

        # ============ Stage C: pairwise carry exchange =================
        contrib = consts.tile([128, NE], F32, tag="contrib")
        nc.vector.tensor_scalar(contrib[:], hcarry[:], mc_sb[:, 0:1], None,
                                op0=OP.mult)
        nc.sync.dma_start(cc_in[:].rearrange("(j p) -> p j", p=128), contrib[:])
        if profile_mode:
            nc.sync.dma_start(cc_out[:], cc_in[:])
        else:
            nc.gpsimd.collective_compute(
                "AllReduce", OP.add,
                replica_groups=[[0, 1], [2, 3], [4, 5], [6, 7]],
                ins=[cc_in[:].opt()], outs=[cc_out[:].opt()])
        craw = consts.tile([128, NE], F32, tag="craw")
        nc.sync.dma_start(craw[:], cc_out[:].rearrange("(j p) -> p j", p=128))
        nc.vector.tensor_scalar(carry[:], craw[:], mu_sb[:, 0:1], None,
                                op0=OP.mult)

        # ============ Stage D: gate proj + correction + out proj =======
        with ExitStack() as sd:
            xs_pool = sd.enter_context(tc.tile_pool(name="xstream2", bufs=10))
            wo = sd.enter_context(tc.tile_pool(name="w_out", bufs=1))
            wo_sb = wo.tile([128, NK, DIM], F32R)
            wo_src = w_out.ap().rearrange("(k p) c -> p k c", p=128)
            gpool = sd.enter_context(tc.tile_pool(name="g", bufs=10))
            hs_pool = sd.enter_context(tc.tile_pool(name="hs", bufs=3))
            ypool = sd.enter_context(tc.tile_pool(name="y", bufs=12))
            opool = sd.enter_context(tc.tile_pool(name="osb", bufs=3))
            for tt in range(NTT):
                xtt = []
                for k in range(NK):
                    if tt == 0 and k < 4:
                        xtt.append(dpre_x[:, k])
                        continue
                    t = xs_pool.tile([128, TT], F32R, tag="xstream2")
                    nc.sync.dma_start(
                        t[:], xT.ap()[k * 128:(k + 1) * 128,
                                      tt * TT:(tt + 1) * TT].bitcast(F32R))
                    xtt.append(t)
                ys = []
                for m in range(NE):
                    pg = ps1.tile([128, TT], F32, tag="ps")
                    for k in range(NK):
                        nc.tensor.matmul(pg[:], wg_in_sb[:, k, m * 128:(m + 1) * 128],
                                         xtt[k][:], start=(k == 0), stop=(k == NK - 1))
                    g = gpool.tile([128, TT], F32, tag="g")
                    nc.scalar.activation(g[:], pg[:], AF.Gelu, bias=c_zero[:])
                    ht = hs_pool.tile([128, TT], F32, tag="hs")
                    nc.sync.dma_start(ht[:], h_dram[m, tt])
                    pt = hs_pool.tile([128, TT], F32, tag="pst")
                    nc.sync.dma_start(pt[:], p_dram[m, tt])
                    htrue = hs_pool.tile([128, TT], F32, tag="htrue")
                    nc.vector.scalar_tensor_tensor(
                        htrue[:], pt[:], carry[:, m:m + 1], ht[:],
                        op0=OP.mult, op1=OP.add)
                    y = ypool.tile([128, TT], F32R, tag="y")
                    nc.vector.tensor_mul(y[:], g[:], htrue[:])
                    ys.append(y)
                    if tt == 0:
                        nc.sync.dma_start(wo_sb[:, m], wo_src[:, m].bitcast(F32R))
                for q in range(TT // 128):
                    po0 = ps1.tile([128, 512], F32, tag="ps")
                    po1 = ps1.tile([128, 512], F32, tag="ps")
                    pos = [po0, po1]
                    for k in range(NE):
                        for n in range(DIM // 512):
                            nc.tensor.matmul(
                                pos[n][:],
                                ys[k][:, q * 128:(q + 1) * 128],
                                wo_sb[:, k, n * 512:(n + 1) * 512],
                                start=(k == 0), stop=(k == NE - 1))
                    osb = opool.tile([128, DIM], F32, tag="osb")
                    for n in range(2):
                        nc.scalar.copy(osb[:, n * 512:(n + 1) * 512], pos[n][:])
                    nc.sync.dma_start(
                        out.ap()[tt * TT + q * 128:tt * TT + (q + 1) * 128, :],
                        osb[:])


_NC_CACHE = {}


def _get_nc():
    if "nc" not in _NC_CACHE:
        _NC_CACHE["nc"] = _build_kernel()
    return _NC_CACHE["nc"]


def _softplus(x):
    return np.logaddexp(0.0, x)


def kernel(x, w_in, w_conv, b_conv, w_gates, b_gates, forget_base, w_out,
           _want_trace=False):
    x = np.asarray(x, dtype=np.float32)
    w_in = np.asarray(w_in, dtype=np.float32)
    w_conv = np.asarray(w_conv, dtype=np.float32)
    b_conv = np.asarray(b_conv, dtype=np.float32)
    w_gates = np.asarray(w_gates, dtype=np.float32)
    b_gates = np.asarray(b_gates, dtype=np.float32)
    forget_base = np.asarray(forget_base, dtype=np.float32)
    w_out = np.asarray(w_out, dtype=np.float32)

    nc = _get_nc()

    w_in_g = np.ascontiguousarray(w_in[:E].T)          # [DIM, E]
    w_in_x = np.ascontiguousarray(w_in[E:].T)          # [DIM, E]
    w_gates_T = np.ascontiguousarray(w_gates.T)        # [E, 2E]
    w_out_T = np.ascontiguousarray(w_out.T)            # [E, DIM]
    wc_r = np.ascontiguousarray(w_conv.reshape(E, KC))
    neg_c = (-8.0 * _softplus(forget_base.astype(np.float64))).astype(
        np.float32)[:, None]
    b_f = b_gates[:E, None].copy()
    b_i = b_gates[E:, None].copy()

    common = {
        "w_in_g": w_in_g, "w_in_x": w_in_x, "w_gates": w_gates_T,
        "w_out": w_out_T, "wc": wc_r, "b_conv": b_conv[:, None].copy(),
        "neg_c": neg_c, "b_f": b_f, "b_i": b_i,
    }
    in_maps = []
    for k in range(N_CORES):
        b, half = k // 2, k % 2
        t0 = half * T_LOC
        xT_loc = np.ascontiguousarray(x[b, t0:t0 + T_LOC, :].T)
        if half == 1:
            # xa for the 3 tokens before this chunk (for the causal conv)
            xa_halo = (x[b, t0 - (KC - 1):t0, :] @ w_in[E:].T).T
            xa_halo = np.ascontiguousarray(xa_halo)
        else:
            xa_halo = np.zeros((E, KC - 1), dtype=np.float32)
        mc = np.full((128, 1), 1.0 if half == 0 else 0.0, dtype=np.float32)
        mu = np.full((128, 1), 0.0 if half == 0 else 1.0, dtype=np.float32)
        in_maps.append({**common, "xT": xT_loc, "xa_halo": xa_halo,
                        "mask_c": mc, "mask_u": mu})

    res = run_bass_kernel_spmd(nc, in_maps, core_ids=list(range(N_CORES)),
                               trace=_want_trace)
    out_full = np.empty((B, T, DIM), dtype=np.float32)
    for k in range(N_CORES):
        b, half = k // 2, k % 2
        out_full[b, half * T_LOC:(half + 1) * T_LOC, :] = res.results[k]["out"]
    if _want_trace:
        return out_full, res
    return out_full



# revision 10
# speedup vs baseline: 1.2628x; 1.2628x over previous
"""Hawk (RG-LRU) block kernel for Trainium2, SPMD over 8 NeuronCores.

Sharding: tokens. Core k handles batch b=k//2, half h=k%2 (2048 tokens).
Fused single pass per 512-token tile: xa-proj + gate-proj (f32r) share one
x stream; causal conv runs in bf16 on DVE; gates GEMM in bf16 (optionally
fp8 DoubleRow); activations batched per ACT table; diagonal recurrence via
hardware tensor_tensor_scan. h/p/gelu spill to DRAM in bf16; pass 2 applies
the cross-half carry correction (4KB pairwise AllReduce) and the f32r
output projection. Gates GEMM is software-pipelined one tile behind the
input projections to keep the PE array continuously fed.
"""
import sys

sys.path.insert(0, "/opt/trn_rl_repo")

import numpy as np
import ml_dtypes
from contextlib import ExitStack

import concourse.bass as bass
import concourse.tile as tile
import concourse.bacc as bacc
from concourse import mybir
from concourse.bass_utils import run_bass_kernel_spmd

F32 = mybir.dt.float32
F32R = mybir.dt.float32r
BF16 = mybir.dt.bfloat16
AF = mybir.ActivationFunctionType
OP = mybir.AluOpType

B, T, DIM = 4, 4096, 1024
E = 1024
KC = 4  # conv taps
N_CORES = 8
T_LOC = T // 2      # 2048 tokens per core
TT = 512            # token tile
NTT = T_LOC // TT   # 4
NE = E // 128       # 8 channel chunks
NK = DIM // 128     # 8 contraction tiles


def _build_kernel(profile_mode=False):
    nc = bacc.Bacc("TRN2", target_bir_lowering=False, debug=False,
                   num_devices=1 if profile_mode else N_CORES)

    xT = nc.dram_tensor("xT", [DIM, T_LOC], BF16, kind="ExternalInput")
    xa_halo = nc.dram_tensor("xa_halo", [E, KC - 1], BF16, kind="ExternalInput")
    w_in_cat = nc.dram_tensor("w_in_cat", [DIM, 2 * E], BF16, kind="ExternalInput")
    w_gates = nc.dram_tensor("w_gates", [E, 2 * E], BF16, kind="ExternalInput")
    w_out = nc.dram_tensor("w_out", [E, DIM], F32, kind="ExternalInput")
    wc = nc.dram_tensor("wc", [E, KC], F32, kind="ExternalInput")
    b_conv = nc.dram_tensor("b_conv", [E, 1], F32, kind="ExternalInput")
    neg_c = nc.dram_tensor("neg_c", [E, 1], F32, kind="ExternalInput")
    nch = nc.dram_tensor("nch", [E, 1], F32, kind="ExternalInput")
    mask_c = nc.dram_tensor("mask_c", [128, 1], F32, kind="ExternalInput")
    mask_u = nc.dram_tensor("mask_u", [128, 1], F32, kind="ExternalInput")
    out = nc.dram_tensor("out", [T_LOC, DIM], BF16, kind="ExternalOutput")

    with tile.TileContext(nc) as tc, ExitStack() as ctx:
        _body(ctx, tc, nc, profile_mode=profile_mode,
              xT=xT, xa_halo=xa_halo, w_in_cat=w_in_cat, w_gates=w_gates,
              w_out=w_out, wc=wc, b_conv=b_conv, neg_c=neg_c, nch=nch,
              mask_c=mask_c, mask_u=mask_u, out=out)
    nc.compile()
    return nc


def _body(ctx, tc, nc, *, xT, xa_halo, w_in_cat, w_gates, w_out, wc,
          b_conv, neg_c, nch, mask_c, mask_u, out, profile_mode=False):
    consts = ctx.enter_context(tc.tile_pool(name="consts", bufs=1))
    dram = ctx.enter_context(tc.tile_pool(name="dram", bufs=1, space="DRAM"))

    def load_chan_const(t_dram, n):
        t = consts.tile([128, NE, n], F32, tag=t_dram.name)
        nc.sync.dma_start(t[:], t_dram.ap().rearrange("(m p) n -> p m n", p=128))
        return t

    wc_sb = load_chan_const(wc, KC)
    b_conv_sb = load_chan_const(b_conv, 1)
    neg_c_sb = load_chan_const(neg_c, 1)
    nch_sb = load_chan_const(nch, 1)
    mc_sb = consts.tile([128, 1], F32, tag="mc")
    nc.sync.dma_start(mc_sb[:], mask_c.ap()[:])
    mu_sb = consts.tile([128, 1], F32, tag="mu")
    nc.sync.dma_start(mu_sb[:], mask_u.ap()[:])
    zeros = consts.tile([128, TT], F32, tag="zeros")
    nc.vector.memset(zeros[:], 0.0)
    c_zero = consts.tile([128, 1], F32, tag="c_zero")
    nc.vector.memset(c_zero[:], 0.0)
    c_sqb = consts.tile([128, 1], F32, tag="c_sqb")
    nc.vector.memset(c_sqb[:], 1.000001)
    hl = consts.tile([128, NE], F32, tag="hl")
    pl = consts.tile([128, NE], F32, tag="pl")
    carry = consts.tile([128, NE], F32, tag="carry")

    h_dram = dram.tile([NTT, 128, NE, TT], BF16, tag="h_spill")
    p_dram = dram.tile([NTT, 128, NE, TT], BF16, tag="p_spill")
    g_dram = dram.tile([NTT, 128, NE, TT], BF16, tag="g_spill")
    cc_in = dram.tile([E], F32, tag="cc_in")
    cc_out = dram.tile([E], F32, tag="cc_out")

    # ---- weights (resident through pass 1) ----
    p1 = ExitStack()
    win_pool = p1.enter_context(tc.tile_pool(name="w_in", bufs=1, side="right"))
    win_sb = win_pool.tile([128, NK, 2 * E], BF16)
    win_src = w_in_cat.ap().rearrange("(k p) f -> p k f", p=128)
    wg_pool = p1.enter_context(tc.tile_pool(name="w_gates", bufs=1, side="right"))
    wg_sb = wg_pool.tile([128, NK, 2 * E], BF16)
    wg_src = w_gates.ap().rearrange("(k p) f -> p k f", p=128)

    xpool = p1.enter_context(tc.tile_pool(name="xs", bufs=2))
    xa_pool = p1.enter_context(tc.tile_pool(name="xa", bufs=2))
    xc_pool = p1.enter_context(tc.tile_pool(name="xc", bufs=2))
    sig_pool = p1.enter_context(tc.tile_pool(name="sig", bufs=8))
    apool = p1.enter_context(tc.tile_pool(name="alpha", bufs=8))
    wpool = p1.enter_context(tc.tile_pool(name="work", bufs=2))
    hout = p1.enter_context(tc.tile_pool(name="hout", bufs=1))
    pout = p1.enter_context(tc.tile_pool(name="pout", bufs=1))
    gout = p1.enter_context(tc.tile_pool(name="gout", bufs=1))
    ps1 = p1.enter_context(tc.tile_pool(name="ps1", bufs=8, space="PSUM"))

    h_t = hout.tile([128, NE, TT], BF16)
    p_t = pout.tile([128, NE, TT], BF16)

    xa_tiles = []   # per-tt xa_ext handles for tail chaining
    xc_tiles = {}   # tt -> xcb tile

    def emit_proj(tt):
        """x load + xa/gate projections + gelu + conv for tile tt."""
        xt = xpool.tile([128, NK, TT], BF16, tag="xt")
        nc.sync.dma_start(
            xt[:],
            xT.ap().rearrange("(k p) t -> p k t", p=128)
            [:, :, tt * TT:(tt + 1) * TT])
        if tt == 0:
            for k in range(NK):
                nc.sync.dma_start(win_sb[:, k], win_src[:, k])
        xa_ext = xa_pool.tile([128, NE, TT + KC - 1], BF16, tag="xa")
        xa_tiles.append(xa_ext)
        if tt == 0:
            nc.sync.dma_start(
                xa_ext[:, :, 0:KC - 1],
                xa_halo.ap().rearrange("(m p) n -> p m n", p=128))
        else:
            nc.vector.tensor_copy(xa_ext[:, :, 0:KC - 1],
                                  xa_tiles[tt - 1][:, :, TT:TT + KC - 1])
        xcb = xc_pool.tile([128, NE, TT], BF16, tag="xcb")
        xc_tiles[tt] = xcb
        for m in range(NE):
            pxa = ps1.tile([128, TT], F32, tag="ps")
            for k in range(NK):
                nc.tensor.matmul(pxa[:], win_sb[:, k, E + m * 128:E + (m + 1) * 128],
                                 xt[:, k], start=(k == 0), stop=(k == NK - 1))
            pg = ps1.tile([128, TT], F32, tag="ps")
            for k in range(NK):
                nc.tensor.matmul(pg[:], win_sb[:, k, m * 128:(m + 1) * 128],
                                 xt[:, k], start=(k == 0), stop=(k == NK - 1))
            nc.scalar.copy(xa_ext[:, m, KC - 1:TT + KC - 1], pxa[:])
            gel = gout.tile([128, TT], BF16, tag="gel", bufs=3)
            nc.scalar.activation(gel[:], pg[:], AF.Gelu, bias=c_zero[:])
            nc.sync.dma_start(g_dram[tt, :, m], gel[:])
            # causal depthwise conv, 4 taps, bf16 accumulate
            nc.vector.tensor_scalar(
                xcb[:, m], xa_ext[:, m, 0:TT], wc_sb[:, m, 0:1],
                b_conv_sb[:, m, 0:1], op0=OP.mult, op1=OP.add)
            for j in range(1, KC):
                nc.vector.scalar_tensor_tensor(
                    xcb[:, m], xa_ext[:, m, j:j + TT], wc_sb[:, m, j:j + 1],
                    xcb[:, m], op0=OP.mult, op1=OP.add)
        if tt == 0:
            for k in range(NK):
                nc.sync.dma_start(wg_sb[:, k], wg_src[:, k])

    def emit_gates(tt):
        """gates GEMM + activations + scans + spills for tile tt."""
        xcb = xc_tiles.pop(tt)
        pfs, pis = [], []
        for m in range(NE):
            pf = ps1.tile([128, TT], F32, tag="ps")
            for k in range(NK):
                nc.tensor.matmul(pf[:], wg_sb[:, k, m * 128:(m + 1) * 128],
                                 xcb[:, k], start=(k == 0), stop=(k == NK - 1))
            pfs.append(pf)
            pi = ps1.tile([128, TT], F32, tag="ps")
            for k in range(NK):
                nc.tensor.matmul(pi[:], wg_sb[:, k, E + m * 128:E + (m + 1) * 128],
                                 xcb[:, k], start=(k == 0), stop=(k == NK - 1))
            pis.append(pi)
        # sigmoid(x) = 0.5*tanh(x/2) + 0.5: Tanh and Exp share an ACT table,
        # so alpha = exp(c*sig(f)) = exp(c/2*tanh(f/2) + c/2) and
        # alpha^2 = exp(c*tanh(f/2) + c) cost zero table switches; only the
        # final Sqrt switches tables.
        sis, alphas, betas = {}, {}, {}
        for m in range(NE):
            thf = wpool.tile([128, TT], F32, tag="thf", bufs=2)
            nc.scalar.activation(thf[:], pfs[m][:], AF.Tanh,
                                 scale=0.5, bias=c_zero[:])
            thi = wpool.tile([128, TT], F32, tag="thi", bufs=2)
            nc.scalar.activation(thi[:], pis[m][:], AF.Tanh,
                                 scale=0.5, bias=c_zero[:])
            alpha = apool.tile([128, TT], F32, tag="alpha")
            nc.scalar.activation(alpha[:], thf[:], AF.Exp,
                                 scale=nch_sb[:, m, 0:1],
                                 bias=nch_sb[:, m, 0:1])
            alphas[m] = alpha
            asq = wpool.tile([128, TT], F32, tag="asq", bufs=8)
            nc.scalar.activation(asq[:], thf[:], AF.Exp,
                                 scale=neg_c_sb[:, m, 0:1],
                                 bias=neg_c_sb[:, m, 0:1])
            betas[m] = asq
            si = sig_pool.tile([128, TT], F32, tag="si")
            nc.vector.tensor_scalar(si[:], thi[:], 0.5, 0.5,
                                    op0=OP.mult, op1=OP.add)
            sis[m] = si
        for m in range(NE):
            nc.scalar.activation(betas[m][:], betas[m][:], AF.Sqrt,
                                 bias=c_sqb[:], scale=-1.0)
        for m in range(NE):
            bsi = wpool.tile([128, TT], F32, tag="bsi", bufs=2)
            nc.gpsimd.tensor_mul(bsi[:], betas[m][:], sis[m][:])
            u = wpool.tile([128, TT], F32, tag="u", bufs=2)
            nc.vector.tensor_mul(u[:], bsi[:], xcb[:, m])
            nc.vector.tensor_tensor_scan(
                h_t[:, m], alphas[m][:], u[:],
                0.0 if tt == 0 else hl[:, m:m + 1],
                op0=OP.mult, op1=OP.add)
            nc.vector.tensor_copy(hl[:, m:m + 1], h_t[:, m, TT - 1:TT])
            nc.vector.tensor_tensor_scan(
                p_t[:, m], alphas[m][:], zeros[:],
                1.0 if tt == 0 else pl[:, m:m + 1],
                op0=OP.mult, op1=OP.add)
            nc.vector.tensor_copy(pl[:, m:m + 1], p_t[:, m, TT - 1:TT])
        nc.sync.dma_start(h_dram[tt], h_t[:])
        nc.sync.dma_start(p_dram[tt], p_t[:])

    # pass 1, gates pipelined one tile behind the projections
    emit_proj(0)
    for tt in range(1, NTT):
        emit_proj(tt)
        emit_gates(tt - 1)
    emit_gates(NTT - 1)

    # ---- carry exchange (4KB pairwise AllReduce) ----
    contrib = consts.tile([128, NE], F32, tag="contrib")
    nc.vector.tensor_scalar(contrib[:], hl[:], mc_sb[:, 0:1], None, op0=OP.mult)
    nc.sync.dma_start(cc_in[:].rearrange("(j p) -> p j", p=128), contrib[:])
    if profile_mode:
        nc.sync.dma_start(cc_out[:], cc_in[:])
    else:
        nc.gpsimd.collective_compute(
            "AllReduce", OP.add,
            replica_groups=[[0, 1], [2, 3], [4, 5], [6, 7]],
            ins=[cc_in[:].opt()], outs=[cc_out[:].opt()])
    craw = consts.tile([128, NE], F32, tag="craw")
    nc.sync.dma_start(craw[:], cc_out[:].rearrange("(j p) -> p j", p=128))
    nc.vector.tensor_scalar(carry[:], craw[:], mu_sb[:, 0:1], None, op0=OP.mult)

    p1.close()

    # ---- pass 2: carry correction + y + output projection ----
    with ExitStack() as p2:
        wo_pool = p2.enter_context(tc.tile_pool(name="w_out", bufs=1, side="right"))
        wo_sb = wo_pool.tile([128, NE, DIM], F32R)
        wo_src = w_out.ap().rearrange("(m p) c -> p m c", p=128)
        for m in range(NE):
            nc.sync.dma_start(wo_sb[:, m], wo_src[:, m].bitcast(F32R))
        gin_pool = p2.enter_context(tc.tile_pool(name="gin", bufs=2))
        hin_pool = p2.enter_context(tc.tile_pool(name="hin", bufs=2))
        pin_pool = p2.enter_context(tc.tile_pool(name="pin", bufs=2))
        ht_pool = p2.enter_context(tc.tile_pool(name="htp", bufs=3))
        y_pool = p2.enter_context(tc.tile_pool(name="y", bufs=9))
        osb_pool = p2.enter_context(tc.tile_pool(name="osb", bufs=3))
        ps2 = p2.enter_context(tc.tile_pool(name="ps2", bufs=8, space="PSUM"))
        for tt in range(NTT):
            gin = gin_pool.tile([128, NE, TT], BF16, tag="gin")
            nc.sync.dma_start(gin[:], g_dram[tt])
            hin = hin_pool.tile([128, NE, TT], BF16, tag="hin")
            nc.sync.dma_start(hin[:], h_dram[tt])
            pin = pin_pool.tile([128, NE, TT], BF16, tag="pin")
            nc.sync.dma_start(pin[:], p_dram[tt])
            ys = []
            for m in range(NE):
                htrue = ht_pool.tile([128, TT], BF16, tag="htrue")
                nc.vector.scalar_tensor_tensor(
                    htrue[:], pin[:, m], carry[:, m:m + 1], hin[:, m],
                    op0=OP.mult, op1=OP.add)
                y = y_pool.tile([128, TT], F32R, tag="y")
                nc.vector.tensor_mul(y[:], gin[:, m], htrue[:])
                ys.append(y)
            for q in range(TT // 128):
                pos = [ps2.tile([128, 512], F32, tag="ps", name=f"po{n}") for n in range(2)]
                for m in range(NE):
                    for n in range(2):
                        nc.tensor.matmul(
                            pos[n][:], ys[m][:, q * 128:(q + 1) * 128],
                            wo_sb[:, m, n * 512:(n + 1) * 512],
                            start=(m == 0), stop=(m == NE - 1))
                osb = osb_pool.tile([128, DIM], BF16, tag="osb")
                for n in range(2):
                    nc.scalar.copy(osb[:, n * 512:(n + 1) * 512], pos[n][:])
                nc.sync.dma_start(
                    out.ap()[tt * TT + q * 128:tt * TT + (q + 1) * 128, :],
                    osb[:])


_NC_CACHE = {}


def _get_nc():
    if "nc" not in _NC_CACHE:
        _NC_CACHE["nc"] = _build_kernel()
    return _NC_CACHE["nc"]


def _softplus(x):
    return np.logaddexp(0.0, x)


def kernel(x, w_in, w_conv, b_conv, w_gates, b_gates, forget_base, w_out,
           _want_trace=False):
    x = np.asarray(x, dtype=np.float32)
    w_in = np.asarray(w_in, dtype=np.float32)
    w_conv = np.asarray(w_conv, dtype=np.float32)
    b_conv = np.asarray(b_conv, dtype=np.float32)
    w_gates = np.asarray(w_gates, dtype=np.float32)
    forget_base = np.asarray(forget_base, dtype=np.float32)
    w_out = np.asarray(w_out, dtype=np.float32)

    nc = _get_nc()

    # [DIM, 2E]: cols 0:E gate-branch, E:2E x-branch
    w_in_cat = np.ascontiguousarray(
        np.concatenate([w_in[:E].T, w_in[E:].T], axis=1)).astype(
            ml_dtypes.bfloat16)
    w_gates_T = np.ascontiguousarray(w_gates.T).astype(ml_dtypes.bfloat16)
    w_out_T = np.ascontiguousarray(w_out.T)            # [E, DIM]
    wc_r = np.ascontiguousarray(w_conv.reshape(E, KC))
    neg_c = (-8.0 * _softplus(forget_base.astype(np.float64))).astype(
        np.float32)[:, None]

    common = {
        "w_in_cat": w_in_cat, "w_gates": w_gates_T, "w_out": w_out_T,
        "wc": wc_r, "b_conv": b_conv[:, None].copy(), "neg_c": neg_c,
        "nch": (0.5 * neg_c).copy(),
    }
    in_maps = []
    for k in range(N_CORES):
        b, half = k // 2, k % 2
        t0 = half * T_LOC
        xT_loc = np.ascontiguousarray(x[b, t0:t0 + T_LOC, :].T).astype(
            ml_dtypes.bfloat16)
        if half == 1:
            # xa for the 3 tokens before this chunk (for the causal conv)
            xa_halo = (x[b, t0 - (KC - 1):t0, :] @ w_in[E:].T).T
            xa_halo = np.ascontiguousarray(xa_halo).astype(ml_dtypes.bfloat16)
        else:
            xa_halo = np.zeros((E, KC - 1), dtype=ml_dtypes.bfloat16)
        mc = np.full((128, 1), 1.0 if half == 0 else 0.0, dtype=np.float32)
        mu = np.full((128, 1), 0.0 if half == 0 else 1.0, dtype=np.float32)
        in_maps.append({**common, "xT": xT_loc, "xa_halo": xa_halo,
                        "mask_c": mc, "mask_u": mu})

    res = run_bass_kernel_spmd(nc, in_maps, core_ids=list(range(N_CORES)),
                               trace=_want_trace)
    out_full = np.empty((B, T, DIM), dtype=np.float32)
    for k in range(N_CORES):
        b, half = k // 2, k % 2
        out_full[b, half * T_LOC:(half + 1) * T_LOC, :] = \
            res.results[k]["out"].astype(np.float32)
    if _want_trace:
        return out_full, res
    return out_full
